# revision 3
# baseline (speedup 1.0000x reference)
"""DetectionLoss Trainium2 kernel — 8 NeuronCores data-parallel.

Split of work:
  * Device (Bass/Tile, SPMD over 8 cores, 8 images each): the dense
    anchor<->gt matching — the arithmetic bulk of this loss (132M anchor-gt
    IoU pairs). Uses the identity  q = inter/((NEG/(1+NEG))(Sa+Sg))  with
    iou = 2q/(7-2q) monotone in q, so best-iou thresholds and argmax reduce
    to per-anchor max of separable outer products ih_g (x) iw_g.  PE computes
    per color-group outer products (block-diagonal K-packed matmuls into
    PSUM), DVE max-accumulates, GPSIMD emits 2-bit mask codes
    (2*(Q>=7/6) + (Q<1)) returned as u8.
  * Host: fp16 iw/ih tables + conflict-graph coloring (so disjoint gt
    windows share one matmul), softplus objectness + per-image top-k hard
    negative mining, and exact sparse loc/cls/obj terms at the ~10k positive
    anchors. Falls back to an exact numpy implementation on any device
    failure.
"""
import os
import sys
import threading
import traceback

import numpy as np

sys.path.insert(0, '/opt/trn_rl_repo')

B, G = 64, 32
NUM_CLASSES = 3
POS_IOU, NEG_IOU, NEG_RATIO = 0.5, 0.4, 3
EPS = 1e-6
SCALES = [(128, 8), (64, 16), (32, 32)]
SIZES = (3.0, 4.0, 5.0)
NCORES = 8
IMGS = B // NCORES
PACKS = [1, 2, 4]
GSTAR = [10, 12, 14]
CAP = 4
BANKS = [[4, 4, 4, 4, 4, 4], [8, 4], [6]]
R_NEG = np.float32(NEG_IOU / (1.0 + NEG_IOU))
Q_POS = float(np.float32(7.0 / 6.0))
CORE_TILES = [IMGS // PACKS[s] * 3 for s in range(3)]
TAB_LEN = sum(CORE_TILES[s] * (CAP * PACKS[s])
              * (GSTAR[s] * 128 + GSTAR[s] * SCALES[s][0]) for s in range(3))
MSK_LEN = IMGS * 3 * sum(W * W for W, _ in SCALES) // 4   # 2-bit packed


# ----------------------------------------------------------------------
# host: tables + coloring
# ----------------------------------------------------------------------

def _prep_tables(gt_boxes):
    """fp16 per-core table blobs. Returns (blobs list[8], ok)."""
    gtb = np.asarray(gt_boxes, np.float32)
    area_g = (gtb[..., 2] - gtb[..., 0]) * (gtb[..., 3] - gtb[..., 1])
    ok = True
    blobs_parts = [[] for _ in range(NCORES)]
    for si, (W, st) in enumerate(SCALES):
        pack = PACKS[si]
        gs = GSTAR[si]
        K = CAP * pack
        sizes = (np.asarray(SIZES, np.float32) * st).astype(np.float32)
        area_a = sizes ** 2
        cols = ((np.arange(W, dtype=np.float32) + 0.5) * st)
        ax1 = cols[None, :] - sizes[:, None] / 2
        ax2 = cols[None, :] + sizes[:, None] / 2
        iw = np.clip(np.minimum(ax2[None, None], gtb[..., 2][..., None, None])
                     - np.maximum(ax1[None, None], gtb[..., 0][..., None, None]),
                     0, None).astype(np.float32)                    # [B,G,A,W]
        ih = np.clip(np.minimum(ax2[None, None], gtb[..., 3][..., None, None])
                     - np.maximum(ax1[None, None], gtb[..., 1][..., None, None]),
                     0, None).astype(np.float32)
        scl = (1.0 / (R_NEG * (area_a[None, None, :] + area_g[..., None])))
        ihs16 = (ih * scl[..., None].astype(np.float32)).astype(np.float16)
        iw16 = iw.astype(np.float16)

        # windows + conflicts per (b,a)
        nzw = iw > 0
        x0 = nzw.argmax(-1); x1 = W - nzw[..., ::-1].argmax(-1)     # [B,G,A]
        nzh = ih > 0
        y0 = nzh.argmax(-1); y1 = W - nzh[..., ::-1].argmax(-1)
        x0t = x0.transpose(0, 2, 1); x1t = x1.transpose(0, 2, 1)
        y0t = y0.transpose(0, 2, 1); y1t = y1.transpose(0, 2, 1)
        xc = (x0t[..., :, None] < x1t[..., None, :]) & (x0t[..., None, :] < x1t[..., :, None])
        yc = (y0t[..., :, None] < y1t[..., None, :]) & (y0t[..., None, :] < y1t[..., :, None])
        conf = (xc & yc)
        idx = np.arange(G)
        conf[..., idx, idx] = False
        deg = conf.sum(-1)
        order = np.argsort(-deg, axis=-1, kind='stable')

        BA = B * 3
        conff = conf.reshape(BA, G, G).astype(np.float32)
        orderf = order.reshape(BA, G)
        member = np.zeros((BA, gs, G), np.float32)
        cnt = np.zeros((BA, gs), np.int32)
        color = np.full((BA, G), -1, np.int64)
        slot = np.full((BA, G), 0, np.int64)
        arange = np.arange(BA)
        for k in range(G):
            g = orderf[:, k]
            cg = conff[arange, g]
            bad = np.einsum('bcG,bG->bc', member, cg) > 0
            feas = ~(bad | (cnt >= CAP))
            load = np.where(feas, cnt, 127)
            pick = load.argmin(1)
            okb = feas[arange, pick]
            if not okb.all():
                ok = False
            color[arange[okb], g[okb]] = pick[okb]
            slot[arange[okb], g[okb]] = cnt[arange[okb], pick[okb]]
            member[arange[okb], pick[okb], g[okb]] = 1.0
            cnt[arange[okb], pick[okb]] += 1
        color = color.reshape(B, 3, G)
        slot = slot.reshape(B, 3, G)

        # scatter straight into merged per-imgtile tables (pack images share
        # a tile; their slot rows are disjoint): [nit, 3, K, gs, 128/W]
        nit = B // pack
        lh_t = np.zeros((nit, 3, K, gs, 128), np.float16)
        rw_t = np.zeros((nit, 3, K, gs, W), np.float16)
        bb, aa, gg = np.nonzero(color >= 0)
        cc = color[bb, aa, gg]
        ss = slot[bb, aa, gg]
        it = bb // pack
        u = bb % pack
        prow = u * CAP + ss
        rw_t[it, aa, prow, cc, :] = iw16[bb, gg, aa, :]
        ihv = ihs16[bb, gg, aa, :]
        for uu in range(pack):
            m = u == uu
            lh_t[it[m], aa[m], prow[m], cc[m], uu * W:(uu + 1) * W] = ihv[m]

        # bank-major blob: per bank: nb lh chunks contiguous, then nb rw
        ct = (nit // NCORES) * 3
        lh_f = lh_t.reshape(NCORES, ct, -1)
        rw_f = rw_t.reshape(NCORES, ct, -1)
        for c in range(NCORES):
            tix = 0
            for nb in BANKS[si]:
                blobs_parts[c].append(lh_f[c, tix:tix + nb].reshape(-1))
                blobs_parts[c].append(rw_f[c, tix:tix + nb].reshape(-1))
                tix += nb
    blobs = [np.concatenate(p) for p in blobs_parts]
    return blobs, ok


# ----------------------------------------------------------------------
# device kernel (Bass/Tile)
# ----------------------------------------------------------------------

def _fix_waits(nc, mybir, maxw=1):
    n = 0
    for f in nc.m.functions:
        for bb in f.blocks:
            insts = bb.instructions
            i = 0
            while i < len(insts):
                ins = insts[i]
                si = ins.sync_info
                waits = list(si.on_wait) if (si and si.on_wait) else []
                if len(waits) > maxw:
                    si.on_wait = waits[:maxw]
                    pos = i
                    for j in range(maxw, len(waits), maxw):
                        n += 1
                        car = mybir.InstDrain(name=f"wc{n}", ins=[], outs=[])
                        car.engine = ins.engine
                        car.sync_info = mybir.SyncInfo(
                            on_wait=waits[j:j + maxw], on_update=[])
                        insts.insert(pos, car)
                        pos += 1
                        i += 1
                i += 1


def _build_nc(fix_waits=True):
    import concourse.bass as bass
    import concourse.mybir as mybir
    from concourse.tile import TileContext

    F16, F32, U8 = mybir.dt.float16, mybir.dt.float32, mybir.dt.uint8
    nc = bass.Bass()
    tab = nc.dram_tensor("tab", [TAB_LEN], F16, kind="ExternalInput")
    msk = nc.dram_tensor("msk", [MSK_LEN], U8, kind="ExternalOutput")

    # bank-major blob: per bank: nb lh chunks [K, gs*128], then nb rw chunks
    bank_off = []
    off = 0
    for s in range(3):
        W = SCALES[s][0]
        gs, K = GSTAR[s], CAP * PACKS[s]
        offs = []
        for nb in BANKS[s]:
            offs.append(off)
            off += nb * K * gs * (128 + W)
        bank_off.append(offs)
    assert off == TAB_LEN
    # bank-major mask blob offsets
    msk_off = []
    mo = 0
    for s in range(3):
        W = SCALES[s][0]
        offs = []
        for nb in BANKS[s]:
            offs.append(mo)
            mo += 128 * (nb * W // 4)
        msk_off.append(offs)
    assert mo == MSK_LEN

    with TileContext(nc) as tc:
        with (tc.tile_pool(name="tabs", bufs=1) as tpool,
              tc.tile_pool(name="acc", bufs=4) as apool,
              tc.tile_pool(name="mtmp", bufs=6) as mpool,
              tc.tile_pool(name="ps", bufs=4, space="PSUM") as ppool):
            bank_tabs = [[] for _ in range(3)]
            for s in range(3):
                W = SCALES[s][0]
                gs, K = GSTAR[s], CAP * PACKS[s]
                tix = 0
                for bi, nb in enumerate(BANKS[s]):
                    nrows, ncols = nb * K, nb * W
                    o = bank_off[s][bi]
                    nl = K * gs * 128
                    nr = K * gs * W
                    blh = tpool.tile([nrows, gs * 128], F16, tag=f"blh{s}_{bi}")
                    brw = tpool.tile([nrows, gs * ncols], F16, tag=f"brw{s}_{bi}")
                    nc.gpsimd.memset(brw[:], 0.0)
                    # one contiguous lh DMA per bank (SP)
                    nc.sync.dma_start(
                        blh[:],
                        tab[o:o + nb * nl].rearrange("(k m) -> k m", k=nrows))
                    # block-diagonal rw placement, issue split ACT/SP
                    for j in range(nb):
                        rw_off = o + nb * nl + j * nr
                        dst = brw[j * K:(j + 1) * K, :].rearrange(
                            "k (g n) -> k g n", g=gs)[:, :, j * W:(j + 1) * W]
                        eng = nc.scalar if j % 2 == 0 else nc.sync
                        eng.dma_start(
                            dst,
                            tab[rw_off:rw_off + K * gs * W]
                            .rearrange("(k g n) -> k g n", k=K, g=gs))
                    bank_tabs[s].append((blh, brw))
                    tix += nb

            for s in range(3):
                W = SCALES[s][0]
                gs, K, pack = GSTAR[s], CAP * PACKS[s], PACKS[s]
                tix = 0
                for bi, nb in enumerate(BANKS[s]):
                    ncols = nb * W
                    ps = ppool.tile([128, ncols], F32)
                    acc = apool.tile([128, ncols], F32)
                    blh, brw = bank_tabs[s][bi]
                    for g in range(gs):
                        nc.tensor.matmul(
                            ps[:],
                            blh[:, g * 128:(g + 1) * 128],
                            brw[:, g * ncols:(g + 1) * ncols],
                            start=True, stop=True)
                        if g == 0:
                            nc.scalar.copy(acc[:], ps[:])
                        else:
                            nc.vector.tensor_max(acc[:], acc[:], ps[:])
                    t1 = mpool.tile([128, ncols], F16, tag="t1")
                    t2 = mpool.tile([128, ncols], F16, tag="t2")
                    pk = mpool.tile([128, ncols // 4], F16, tag="pk")
                    m8 = mpool.tile([128, ncols // 4], U8, tag="m8")
                    nc.gpsimd.tensor_scalar(
                        t1[:], acc[:], Q_POS, 2.0,
                        op0=mybir.AluOpType.is_ge, op1=mybir.AluOpType.mult)
                    nc.gpsimd.tensor_scalar(
                        t2[:], acc[:], 1.0, None, op0=mybir.AluOpType.is_lt)
                    nc.gpsimd.tensor_tensor(
                        t1[:], t1[:], t2[:], op=mybir.AluOpType.add)
                    # pack 4 neighbouring 2-bit codes into one byte
                    # (strided APs -> DVE; GPSIMD only handles contiguous)
                    tq = t1[:].rearrange("p (q f) -> p q f", f=4)
                    tmp = t2[:, :ncols // 4]
                    nc.vector.tensor_scalar(pk[:], tq[:, :, 1], 4.0, None,
                                            op0=mybir.AluOpType.mult)
                    nc.vector.tensor_tensor(pk[:], pk[:], tq[:, :, 0],
                                            op=mybir.AluOpType.add)
                    nc.vector.tensor_scalar(tmp, tq[:, :, 2], 16.0, None,
                                            op0=mybir.AluOpType.mult)
                    nc.vector.tensor_tensor(pk[:], pk[:], tmp,
                                            op=mybir.AluOpType.add)
                    nc.vector.tensor_scalar(tmp, tq[:, :, 3], 64.0, None,
                                            op0=mybir.AluOpType.mult)
                    nc.vector.tensor_tensor(pk[:], pk[:], tmp,
                                            op=mybir.AluOpType.add)
                    nc.gpsimd.tensor_copy(m8[:], pk[:])
                    # one contiguous mask DMA per bank (bank-major blob)
                    mo = msk_off[s][bi]
                    nc.sync.dma_start(
                        msk[mo:mo + 128 * (ncols // 4)]
                        .rearrange("(p m) -> p m", p=128),
                        m8[:])
                    tix += nb
    if fix_waits:
        _fix_waits(nc, mybir)
    return nc


_DEV = {"nc": None, "err": None, "warm": False}
_DEV_LOCK = threading.Lock()


def _ensure_device(warm_inputs=None):
    with _DEV_LOCK:
        if _DEV["nc"] is None:
            _DEV["nc"] = _build_nc()
        if not _DEV["warm"]:
            from concourse.bass_utils import run_bass_kernel_spmd
            blobs = (warm_inputs if warm_inputs is not None
                     else [np.zeros(TAB_LEN, np.float16)] * NCORES)
            res = run_bass_kernel_spmd(
                _DEV["nc"], [{"tab": b} for b in blobs],
                core_ids=list(range(NCORES)))
            _DEV["warm"] = True
            return res
    return None


def _run_device(blobs):
    from concourse.bass_utils import run_bass_kernel_spmd
    res = run_bass_kernel_spmd(
        _DEV["nc"], [{"tab": b} for b in blobs], core_ids=list(range(NCORES)))
    return [np.asarray(res.results[c]["msk"]) for c in range(NCORES)]


# ----------------------------------------------------------------------
# host: finishing from device masks
# ----------------------------------------------------------------------

def _finish(preds, anchors, gtb, gtl, mask_blobs, sps=None):
    total = np.float64(0.0)
    s0 = IMGS * 3 * 128 * 128 // 4
    s1 = IMGS * 3 * 64 * 64 // 4
    s2 = IMGS * 3 * 32 * 32 // 4
    offs = [0, s0, s0 + s1]
    lens = [s0, s1, s2]
    for si, (W, st) in enumerate(SCALES):
        anc = anchors[si]
        pred = preds[si]
        # bank-major device layout: per bank [128, nb*W/4] packed bytes;
        # subtile j of a bank is global tile gt (it*3+a), partitions are
        # `pack` image bands of W rows.
        pack = PACKS[si]
        m = np.empty((B, 3, W, W), np.uint8)
        for c in range(NCORES):
            o = offs[si]
            parts = []
            for nb in BANKS[si]:
                n = 128 * (nb * W // 4)
                pkc = mask_blobs[c][o:o + n].reshape(128, nb * W // 4)
                o += n
                t = np.empty(pkc.shape + (4,), np.uint8)
                t[..., 0] = pkc & 3
                t[..., 1] = (pkc >> 2) & 3
                t[..., 2] = (pkc >> 4) & 3
                t[..., 3] = pkc >> 6
                # [128, nb, W] -> [nb(gt), 128, W]
                parts.append(t.reshape(128, nb, W).transpose(1, 0, 2))
            arr = np.concatenate(parts)          # [ct, pack*W, W], gt-major
            nit = IMGS // pack
            m[c * IMGS:(c + 1) * IMGS] = (
                arr.reshape(nit, 3, pack, W, W)
                .transpose(0, 2, 1, 3, 4).reshape(IMGS, 3, W, W))
        pm = (m & 2) != 0
        nm = (m & 1) != 0

        x4 = pred[:, 4::8]                       # [B,3,H,W] objectness logits
        sp = (sps[si] if sps is not None
              else np.logaddexp(np.float32(0.0), x4))
        npos = pm.reshape(B, -1).sum(1)
        nneg = nm.reshape(B, -1).sum(1)
        cand = np.where(nm, sp, np.float32(-1e9)).reshape(B, -1)
        N = cand.shape[1]
        ks = np.minimum(NEG_RATIO * np.maximum(1, npos), nneg)
        kmax = max(1, int(ks.max()))
        top = np.partition(cand, N - kmax, axis=1)[:, N - kmax:]
        top.sort(axis=1)
        cs = np.cumsum(top[:, ::-1], axis=1, dtype=np.float64)
        sel = cs[np.arange(B), np.maximum(ks, 1) - 1]
        total += np.float64(np.where(ks > 0, sel, 0.0).sum())

        bb, aa, yy, xx = np.nonzero(pm)
        if bb.size:
            n_id = (yy * W + xx) * 3 + aa
            pa = anc[n_id]
            gb = gtb[bb]
            lt = np.maximum(pa[:, None, :2], gb[..., :2])
            rb = np.minimum(pa[:, None, 2:], gb[..., 2:])
            wh = np.clip(rb - lt, 0, None)
            inter = wh[..., 0] * wh[..., 1]
            area_a = (pa[:, 2] - pa[:, 0]) * (pa[:, 3] - pa[:, 1])
            area_b = (gb[..., 2] - gb[..., 0]) * (gb[..., 3] - gb[..., 1])
            iou = inter / (area_a[:, None] + area_b - inter + np.float32(1e-9))
            bidx = iou.argmax(1)
            mb = gtb[bb, bidx]
            axc = (pa[:, 0] + pa[:, 2]) * 0.5
            ayc = (pa[:, 1] + pa[:, 3]) * 0.5
            aw = np.maximum(pa[:, 2] - pa[:, 0], np.float32(EPS))
            ah = np.maximum(pa[:, 3] - pa[:, 1], np.float32(EPS))
            gx = (mb[:, 0] + mb[:, 2]) * 0.5
            gy = (mb[:, 1] + mb[:, 3]) * 0.5
            gw = np.maximum(mb[:, 2] - mb[:, 0], np.float32(EPS))
            gh = np.maximum(mb[:, 3] - mb[:, 1], np.float32(EPS))
            tt = np.stack([(gx - axc) / aw, (gy - ayc) / ah,
                           np.log(gw / aw), np.log(gh / ah)], 1)
            pv = pred[bb[:, None], (aa[:, None] * 8 + np.arange(8)[None, :]),
                      yy[:, None], xx[:, None]]
            d = pv[:, :4] - tt
            adx = np.abs(d)
            sl1 = np.where(adx < 1, np.float32(0.5) * d * d,
                           adx - np.float32(0.5))
            total += np.float64(sl1.sum(dtype=np.float64))
            xv = pv[:, 4]
            total += np.float64(
                (np.logaddexp(np.float32(0.0), xv) - xv).sum(dtype=np.float64))
            logits = pv[:, 5:]
            mx = logits.max(1)
            lse = mx + np.log(np.exp(logits - mx[:, None]).sum(1))
            lab = np.maximum(gtl[bb, bidx], 0)
            ce = lse - logits[np.arange(len(bb)), lab]
            total += np.float64(ce.sum(dtype=np.float64))
    return np.float32(total / B)


# ----------------------------------------------------------------------
# exact numpy fallback (no device)
# ----------------------------------------------------------------------

def _numpy_kernel(preds, anchors, gtb, gtl):
    total = np.float64(0.0)
    for si in range(3):
        anc = anchors[si]
        N = anc.shape[0]
        p_all = preds[si].transpose(0, 2, 3, 1).reshape(B, N, 8)
        for b in range(B):
            p = p_all[b]
            a = anc
            gb = gtb[b]
            lt = np.maximum(a[:, None, :2], gb[None, :, :2])
            rb = np.minimum(a[:, None, 2:], gb[None, :, 2:])
            wh = np.clip(rb - lt, 0, None)
            inter = wh[..., 0] * wh[..., 1]
            area_a = (a[:, 2] - a[:, 0]) * (a[:, 3] - a[:, 1])
            area_b = (gb[:, 2] - gb[:, 0]) * (gb[:, 3] - gb[:, 1])
            iou = inter / (area_a[:, None] + area_b[None, :] - inter
                           + np.float32(1e-9))
            best = iou.max(1)
            bidx = iou.argmax(1)
            pos = best >= POS_IOU
            neg = best < NEG_IOU
            posf = pos.astype(np.float32)
            mb = gb[bidx]
            axc = (a[:, 0] + a[:, 2]) * 0.5
            ayc = (a[:, 1] + a[:, 3]) * 0.5
            aw = np.maximum(a[:, 2] - a[:, 0], np.float32(EPS))
            ah = np.maximum(a[:, 3] - a[:, 1], np.float32(EPS))
            gx = (mb[:, 0] + mb[:, 2]) * 0.5
            gy = (mb[:, 1] + mb[:, 3]) * 0.5
            gw = np.maximum(mb[:, 2] - mb[:, 0], np.float32(EPS))
            gh = np.maximum(mb[:, 3] - mb[:, 1], np.float32(EPS))
            t = [(gx - axc) / aw, (gy - ayc) / ah,
                 np.log(gw / aw), np.log(gh / ah)]
            def sl1(x):
                ax_ = np.abs(x)
                return np.where(ax_ < 1, np.float32(0.5) * x * x,
                                ax_ - np.float32(0.5))
            total += np.float64((posf * (sl1(p[:, 0] - t[0]) + sl1(p[:, 1] - t[1])
                                 + sl1(p[:, 2] - t[2]) + sl1(p[:, 3] - t[3])
                                 )).sum(dtype=np.float64))
            x = p[:, 4]
            obj_all = (np.maximum(x, 0) - x * posf
                       + np.log1p(np.exp(-np.abs(x))))
            num_keep = NEG_RATIO * max(1, int(pos.sum()))
            neg_loss = np.where(neg, obj_all, np.float32(-1e9))
            order = np.argsort(-neg_loss, kind='stable')
            ranks = np.empty(N, np.int64)
            ranks[order] = np.arange(N)
            selected = neg & (ranks < num_keep)
            total += np.float64(
                (obj_all * (posf + selected)).sum(dtype=np.float64))
            mx = p[:, 5:].max(1)
            lse = mx + np.log(np.exp(p[:, 5:] - mx[:, None]).sum(1))
            ce = lse - p[np.arange(N), 5 + np.maximum(gtl[b][bidx], 0)]
            total += np.float64((posf * ce).sum(dtype=np.float64))
    return np.float32(total / B)


# ----------------------------------------------------------------------

def kernel(pred0, pred1, pred2, anchors0, anchors1, anchors2,
           gt_boxes, gt_labels):
    preds = [np.asarray(p, dtype=np.float32) for p in (pred0, pred1, pred2)]
    anchors = [np.asarray(a, dtype=np.float32)
               for a in (anchors0, anchors1, anchors2)]
    gtb = np.asarray(gt_boxes, dtype=np.float32)
    gtl = np.asarray(gt_labels)
    import time as _time
    tmr = {}
    try:
        t0 = _time.time()
        blobs, ok = _prep_tables(gtb)
        tmr['prep'] = _time.time() - t0
        if not ok:
            raise RuntimeError("coloring overflow; fallback")
        t0 = _time.time()
        _ensure_device()
        tmr['ensure'] = _time.time() - t0
        box = {}

        def dev():
            try:
                t = _time.time()
                box['masks'] = _run_device(blobs)
                tmr['device'] = _time.time() - t
            except Exception as e:   # noqa: BLE001
                box['err'] = e
        th = threading.Thread(target=dev, daemon=True)
        t0 = _time.time()
        th.start()
        # overlap: softplus objectness per scale (device-independent)
        sps = [np.logaddexp(np.float32(0.0), preds[si][:, 4::8])
               for si in range(3)]
        tmr['sp_overlap'] = _time.time() - t0
        th.join(timeout=90.0)
        tmr['dev_wall'] = _time.time() - t0
        if th.is_alive():
            raise RuntimeError("device run timed out")
        if 'err' in box:
            raise box['err']
        t0 = _time.time()
        r = _finish(preds, anchors, gtb, gtl, box['masks'], sps)
        tmr['finish'] = _time.time() - t0
        if os.environ.get("DETLOSS_TIMERS"):
            print("timers:", {k: round(v, 3) for k, v in tmr.items()},
                  file=sys.stderr)
        return r
    except Exception:
        traceback.print_exc()
        return _numpy_kernel(preds, anchors, gtb, gtl)


# import-time warmup (compile-cache hit + jit executable load) unless told no
if not os.environ.get("DETLOSS_NO_WARMUP"):
    try:
        _ensure_device()
    except Exception:   # noqa: BLE001
        traceback.print_exc()


# revision 4
# speedup vs baseline: 1.6533x; 1.6533x over previous
"""DetectionLoss Trainium2 kernel — 8 NeuronCores data-parallel.

Split of work:
  * Device (Bass/Tile, SPMD over 8 cores, 8 images each): the dense
    anchor<->gt matching — the arithmetic bulk of this loss (132M anchor-gt
    IoU pairs). Uses the identity  q = inter/((NEG/(1+NEG))(Sa+Sg))  with
    iou = 2q/(7-2q) monotone in q, so best-iou thresholds and argmax reduce
    to per-anchor max of separable outer products ih_g (x) iw_g.  PE computes
    per color-group outer products (block-diagonal K-packed matmuls into
    PSUM), DVE max-accumulates, GPSIMD emits 2-bit mask codes
    (2*(Q>=7/6) + (Q<1)) returned as u8.
  * Host: fp16 iw/ih tables + conflict-graph coloring (so disjoint gt
    windows share one matmul), softplus objectness + per-image top-k hard
    negative mining, and exact sparse loc/cls/obj terms at the ~10k positive
    anchors. Falls back to an exact numpy implementation on any device
    failure.
"""
import os
import sys
import threading
import traceback

import numpy as np

sys.path.insert(0, '/opt/trn_rl_repo')

B, G = 64, 32
NUM_CLASSES = 3
POS_IOU, NEG_IOU, NEG_RATIO = 0.5, 0.4, 3
EPS = 1e-6
SCALES = [(128, 8), (64, 16), (32, 32)]
SIZES = (3.0, 4.0, 5.0)
NCORES = 8
IMGS = B // NCORES
PACKS = [1, 2, 4]
GSTAR = [10, 12, 14]
CAP = 4
BANKS = [[4, 4, 4, 4, 4, 4], [8, 4], [6]]
R_NEG = np.float32(NEG_IOU / (1.0 + NEG_IOU))
Q_POS = float(np.float32(7.0 / 6.0))
CORE_TILES = [IMGS // PACKS[s] * 3 for s in range(3)]
TAB_LEN = sum(CORE_TILES[s] * (CAP * PACKS[s])
              * (GSTAR[s] * 128 + GSTAR[s] * SCALES[s][0]) for s in range(3))
MSK_LEN = IMGS * 3 * sum(W * W for W, _ in SCALES) // 4   # 2-bit packed


# ----------------------------------------------------------------------
# host: tables + coloring
# ----------------------------------------------------------------------

def _prep_tables(gt_boxes):
    """fp16 per-core table blobs. Returns (blobs list[8], ok)."""
    gtb = np.asarray(gt_boxes, np.float32)
    area_g = (gtb[..., 2] - gtb[..., 0]) * (gtb[..., 3] - gtb[..., 1])
    ok = True
    blobs_parts = [[] for _ in range(NCORES)]
    for si, (W, st) in enumerate(SCALES):
        pack = PACKS[si]
        gs = GSTAR[si]
        K = CAP * pack
        sizes = (np.asarray(SIZES, np.float32) * st).astype(np.float32)
        area_a = sizes ** 2
        cols = ((np.arange(W, dtype=np.float32) + 0.5) * st)
        ax1 = cols[None, :] - sizes[:, None] / 2
        ax2 = cols[None, :] + sizes[:, None] / 2
        iw = np.clip(np.minimum(ax2[None, None], gtb[..., 2][..., None, None])
                     - np.maximum(ax1[None, None], gtb[..., 0][..., None, None]),
                     0, None).astype(np.float32)                    # [B,G,A,W]
        ih = np.clip(np.minimum(ax2[None, None], gtb[..., 3][..., None, None])
                     - np.maximum(ax1[None, None], gtb[..., 1][..., None, None]),
                     0, None).astype(np.float32)
        scl = (1.0 / (R_NEG * (area_a[None, None, :] + area_g[..., None])))
        ihs16 = (ih * scl[..., None].astype(np.float32)).astype(np.float16)
        iw16 = iw.astype(np.float16)

        # windows + conflicts per (b,a)
        nzw = iw > 0
        x0 = nzw.argmax(-1); x1 = W - nzw[..., ::-1].argmax(-1)     # [B,G,A]
        nzh = ih > 0
        y0 = nzh.argmax(-1); y1 = W - nzh[..., ::-1].argmax(-1)
        x0t = x0.transpose(0, 2, 1); x1t = x1.transpose(0, 2, 1)
        y0t = y0.transpose(0, 2, 1); y1t = y1.transpose(0, 2, 1)
        xc = (x0t[..., :, None] < x1t[..., None, :]) & (x0t[..., None, :] < x1t[..., :, None])
        yc = (y0t[..., :, None] < y1t[..., None, :]) & (y0t[..., None, :] < y1t[..., :, None])
        conf = (xc & yc)
        idx = np.arange(G)
        conf[..., idx, idx] = False
        deg = conf.sum(-1)
        order = np.argsort(-deg, axis=-1, kind='stable')

        BA = B * 3
        conff = conf.reshape(BA, G, G).astype(np.float32)
        orderf = order.reshape(BA, G)
        member = np.zeros((BA, gs, G), np.float32)
        cnt = np.zeros((BA, gs), np.int32)
        color = np.full((BA, G), -1, np.int64)
        slot = np.full((BA, G), 0, np.int64)
        arange = np.arange(BA)
        for k in range(G):
            g = orderf[:, k]
            cg = conff[arange, g]
            bad = np.einsum('bcG,bG->bc', member, cg) > 0
            feas = ~(bad | (cnt >= CAP))
            load = np.where(feas, cnt, 127)
            pick = load.argmin(1)
            okb = feas[arange, pick]
            if not okb.all():
                ok = False
            color[arange[okb], g[okb]] = pick[okb]
            slot[arange[okb], g[okb]] = cnt[arange[okb], pick[okb]]
            member[arange[okb], pick[okb], g[okb]] = 1.0
            cnt[arange[okb], pick[okb]] += 1
        color = color.reshape(B, 3, G)
        slot = slot.reshape(B, 3, G)

        # scatter straight into merged per-imgtile tables (pack images share
        # a tile; their slot rows are disjoint): [nit, 3, K, gs, 128/W]
        nit = B // pack
        lh_t = np.zeros((nit, 3, K, gs, 128), np.float16)
        rw_t = np.zeros((nit, 3, K, gs, W), np.float16)
        bb, aa, gg = np.nonzero(color >= 0)
        cc = color[bb, aa, gg]
        ss = slot[bb, aa, gg]
        it = bb // pack
        u = bb % pack
        prow = u * CAP + ss
        rw_t[it, aa, prow, cc, :] = iw16[bb, gg, aa, :]
        ihv = ihs16[bb, gg, aa, :]
        for uu in range(pack):
            m = u == uu
            lh_t[it[m], aa[m], prow[m], cc[m], uu * W:(uu + 1) * W] = ihv[m]

        # bank-major blob: per bank: nb lh chunks contiguous, then nb rw
        ct = (nit // NCORES) * 3
        lh_f = lh_t.reshape(NCORES, ct, -1)
        rw_f = rw_t.reshape(NCORES, ct, -1)
        for c in range(NCORES):
            tix = 0
            for nb in BANKS[si]:
                blobs_parts[c].append(lh_f[c, tix:tix + nb].reshape(-1))
                blobs_parts[c].append(rw_f[c, tix:tix + nb].reshape(-1))
                tix += nb
    blobs = [np.concatenate(p) for p in blobs_parts]
    return blobs, ok


# ----------------------------------------------------------------------
# device kernel (Bass/Tile)
# ----------------------------------------------------------------------

def _fix_waits(nc, mybir, maxw=1):
    n = 0
    for f in nc.m.functions:
        for bb in f.blocks:
            insts = bb.instructions
            i = 0
            while i < len(insts):
                ins = insts[i]
                si = ins.sync_info
                waits = list(si.on_wait) if (si and si.on_wait) else []
                if len(waits) > maxw:
                    si.on_wait = waits[:maxw]
                    pos = i
                    for j in range(maxw, len(waits), maxw):
                        n += 1
                        car = mybir.InstDrain(name=f"wc{n}", ins=[], outs=[])
                        car.engine = ins.engine
                        car.sync_info = mybir.SyncInfo(
                            on_wait=waits[j:j + maxw], on_update=[])
                        insts.insert(pos, car)
                        pos += 1
                        i += 1
                i += 1


def _build_nc(fix_waits=True):
    import concourse.bass as bass
    import concourse.mybir as mybir
    from concourse.tile import TileContext

    F16, F32, U8 = mybir.dt.float16, mybir.dt.float32, mybir.dt.uint8
    nc = bass.Bass()
    tab = nc.dram_tensor("tab", [TAB_LEN], F16, kind="ExternalInput")
    msk = nc.dram_tensor("msk", [MSK_LEN], U8, kind="ExternalOutput")

    # bank-major blob: per bank: nb lh chunks [K, gs*128], then nb rw chunks
    bank_off = []
    off = 0
    for s in range(3):
        W = SCALES[s][0]
        gs, K = GSTAR[s], CAP * PACKS[s]
        offs = []
        for nb in BANKS[s]:
            offs.append(off)
            off += nb * K * gs * (128 + W)
        bank_off.append(offs)
    assert off == TAB_LEN
    # bank-major mask blob offsets
    msk_off = []
    mo = 0
    for s in range(3):
        W = SCALES[s][0]
        offs = []
        for nb in BANKS[s]:
            offs.append(mo)
            mo += 128 * (nb * W // 4)
        msk_off.append(offs)
    assert mo == MSK_LEN

    with TileContext(nc) as tc:
        with (tc.tile_pool(name="tabs", bufs=1) as tpool,
              tc.tile_pool(name="acc", bufs=4) as apool,
              tc.tile_pool(name="mtmp", bufs=6) as mpool,
              tc.tile_pool(name="ps", bufs=4, space="PSUM") as ppool):
            bank_tabs = [[] for _ in range(3)]
            for s in range(3):
                W = SCALES[s][0]
                gs, K = GSTAR[s], CAP * PACKS[s]
                tix = 0
                for bi, nb in enumerate(BANKS[s]):
                    nrows, ncols = nb * K, nb * W
                    o = bank_off[s][bi]
                    nl = K * gs * 128
                    nr = K * gs * W
                    blh = tpool.tile([nrows, gs * 128], F16, tag=f"blh{s}_{bi}")
                    brw = tpool.tile([nrows, gs * ncols], F16, tag=f"brw{s}_{bi}")
                    nc.gpsimd.memset(brw[:], 0.0)
                    # one contiguous lh DMA per bank (SP)
                    nc.sync.dma_start(
                        blh[:],
                        tab[o:o + nb * nl].rearrange("(k m) -> k m", k=nrows))
                    # block-diagonal rw placement, issue split ACT/SP
                    for j in range(nb):
                        rw_off = o + nb * nl + j * nr
                        dst = brw[j * K:(j + 1) * K, :].rearrange(
                            "k (g n) -> k g n", g=gs)[:, :, j * W:(j + 1) * W]
                        eng = nc.scalar if j % 2 == 0 else nc.sync
                        eng.dma_start(
                            dst,
                            tab[rw_off:rw_off + K * gs * W]
                            .rearrange("(k g n) -> k g n", k=K, g=gs))
                    bank_tabs[s].append((blh, brw))
                    tix += nb

            for s in range(3):
                W = SCALES[s][0]
                gs, K, pack = GSTAR[s], CAP * PACKS[s], PACKS[s]
                tix = 0
                for bi, nb in enumerate(BANKS[s]):
                    ncols = nb * W
                    ps = ppool.tile([128, ncols], F32)
                    acc = apool.tile([128, ncols], F32)
                    blh, brw = bank_tabs[s][bi]
                    for g in range(gs):
                        nc.tensor.matmul(
                            ps[:],
                            blh[:, g * 128:(g + 1) * 128],
                            brw[:, g * ncols:(g + 1) * ncols],
                            start=True, stop=True)
                        if g == 0:
                            nc.scalar.copy(acc[:], ps[:])
                        else:
                            nc.vector.tensor_max(acc[:], acc[:], ps[:])
                    t1 = mpool.tile([128, ncols], F16, tag="t1")
                    t2 = mpool.tile([128, ncols], F16, tag="t2")
                    pk = mpool.tile([128, ncols // 4], F16, tag="pk")
                    m8 = mpool.tile([128, ncols // 4], U8, tag="m8")
                    nc.gpsimd.tensor_scalar(
                        t1[:], acc[:], Q_POS, 2.0,
                        op0=mybir.AluOpType.is_ge, op1=mybir.AluOpType.mult)
                    nc.gpsimd.tensor_scalar(
                        t2[:], acc[:], 1.0, None, op0=mybir.AluOpType.is_lt)
                    nc.gpsimd.tensor_tensor(
                        t1[:], t1[:], t2[:], op=mybir.AluOpType.add)
                    # pack 4 neighbouring 2-bit codes into one byte, fused
                    # (in0*s) op in1 per op; strided APs -> DVE only
                    tq = t1[:].rearrange("p (q f) -> p q f", f=4)
                    tmp = t2[:, :ncols // 4]
                    nc.vector.scalar_tensor_tensor(
                        pk[:], tq[:, :, 1], 4.0, tq[:, :, 0],
                        op0=mybir.AluOpType.mult, op1=mybir.AluOpType.add)
                    nc.vector.scalar_tensor_tensor(
                        tmp, tq[:, :, 3], 4.0, tq[:, :, 2],
                        op0=mybir.AluOpType.mult, op1=mybir.AluOpType.add)
                    nc.vector.scalar_tensor_tensor(
                        pk[:], tmp, 16.0, pk[:],
                        op0=mybir.AluOpType.mult, op1=mybir.AluOpType.add)
                    nc.gpsimd.tensor_copy(m8[:], pk[:])
                    # one contiguous mask DMA per bank (bank-major blob)
                    mo = msk_off[s][bi]
                    nc.sync.dma_start(
                        msk[mo:mo + 128 * (ncols // 4)]
                        .rearrange("(p m) -> p m", p=128),
                        m8[:])
                    tix += nb
    if fix_waits:
        _fix_waits(nc, mybir)
    return nc


_DEV = {"nc": None, "err": None, "warm": False}
_DEV_LOCK = threading.Lock()


def _ensure_device(warm_inputs=None):
    with _DEV_LOCK:
        if _DEV["nc"] is None:
            _DEV["nc"] = _build_nc()
        if not _DEV["warm"]:
            from concourse.bass_utils import run_bass_kernel_spmd
            blobs = (warm_inputs if warm_inputs is not None
                     else [np.zeros(TAB_LEN, np.float16)] * NCORES)
            res = run_bass_kernel_spmd(
                _DEV["nc"], [{"tab": b} for b in blobs],
                core_ids=list(range(NCORES)))
            _DEV["warm"] = True
            return res
    return None


def _run_device(blobs):
    from concourse.bass_utils import run_bass_kernel_spmd
    res = run_bass_kernel_spmd(
        _DEV["nc"], [{"tab": b} for b in blobs], core_ids=list(range(NCORES)))
    return [np.asarray(res.results[c]["msk"]) for c in range(NCORES)]


# ----------------------------------------------------------------------
# host: finishing from device masks
# ----------------------------------------------------------------------

def _finish(preds, anchors, gtb, gtl, mask_blobs, sps=None):
    total = np.float64(0.0)
    s0 = IMGS * 3 * 128 * 128 // 4
    s1 = IMGS * 3 * 64 * 64 // 4
    s2 = IMGS * 3 * 32 * 32 // 4
    offs = [0, s0, s0 + s1]
    lens = [s0, s1, s2]
    for si, (W, st) in enumerate(SCALES):
        anc = anchors[si]
        pred = preds[si]
        # bank-major device layout: per bank [128, nb*W/4] packed bytes;
        # subtile j of a bank is global tile gt (it*3+a), partitions are
        # `pack` image bands of W rows.
        pack = PACKS[si]
        m = np.empty((B, 3, W, W), np.uint8)
        for c in range(NCORES):
            o = offs[si]
            parts = []
            for nb in BANKS[si]:
                n = 128 * (nb * W // 4)
                pkc = mask_blobs[c][o:o + n].reshape(128, nb * W // 4)
                o += n
                t = np.empty(pkc.shape + (4,), np.uint8)
                t[..., 0] = pkc & 3
                t[..., 1] = (pkc >> 2) & 3
                t[..., 2] = (pkc >> 4) & 3
                t[..., 3] = pkc >> 6
                # [128, nb, W] -> [nb(gt), 128, W]
                parts.append(t.reshape(128, nb, W).transpose(1, 0, 2))
            arr = np.concatenate(parts)          # [ct, pack*W, W], gt-major
            nit = IMGS // pack
            m[c * IMGS:(c + 1) * IMGS] = (
                arr.reshape(nit, 3, pack, W, W)
                .transpose(0, 2, 1, 3, 4).reshape(IMGS, 3, W, W))
        pm = (m & 2) != 0
        nm = (m & 1) != 0

        x4 = pred[:, 4::8]                       # [B,3,H,W] objectness logits
        sp = (sps[si] if sps is not None
              else np.logaddexp(np.float32(0.0), x4))
        npos = pm.reshape(B, -1).sum(1)
        nneg = nm.reshape(B, -1).sum(1)
        cand = np.where(nm, sp, np.float32(-1e9)).reshape(B, -1)
        N = cand.shape[1]
        ks = np.minimum(NEG_RATIO * np.maximum(1, npos), nneg)
        kmax = max(1, int(ks.max()))
        top = np.partition(cand, N - kmax, axis=1)[:, N - kmax:]
        top.sort(axis=1)
        cs = np.cumsum(top[:, ::-1], axis=1, dtype=np.float64)
        sel = cs[np.arange(B), np.maximum(ks, 1) - 1]
        total += np.float64(np.where(ks > 0, sel, 0.0).sum())

        bb, aa, yy, xx = np.nonzero(pm)
        if bb.size:
            n_id = (yy * W + xx) * 3 + aa
            pa = anc[n_id]
            gb = gtb[bb]
            lt = np.maximum(pa[:, None, :2], gb[..., :2])
            rb = np.minimum(pa[:, None, 2:], gb[..., 2:])
            wh = np.clip(rb - lt, 0, None)
            inter = wh[..., 0] * wh[..., 1]
            area_a = (pa[:, 2] - pa[:, 0]) * (pa[:, 3] - pa[:, 1])
            area_b = (gb[..., 2] - gb[..., 0]) * (gb[..., 3] - gb[..., 1])
            iou = inter / (area_a[:, None] + area_b - inter + np.float32(1e-9))
            bidx = iou.argmax(1)
            mb = gtb[bb, bidx]
            axc = (pa[:, 0] + pa[:, 2]) * 0.5
            ayc = (pa[:, 1] + pa[:, 3]) * 0.5
            aw = np.maximum(pa[:, 2] - pa[:, 0], np.float32(EPS))
            ah = np.maximum(pa[:, 3] - pa[:, 1], np.float32(EPS))
            gx = (mb[:, 0] + mb[:, 2]) * 0.5
            gy = (mb[:, 1] + mb[:, 3]) * 0.5
            gw = np.maximum(mb[:, 2] - mb[:, 0], np.float32(EPS))
            gh = np.maximum(mb[:, 3] - mb[:, 1], np.float32(EPS))
            tt = np.stack([(gx - axc) / aw, (gy - ayc) / ah,
                           np.log(gw / aw), np.log(gh / ah)], 1)
            pv = pred[bb[:, None], (aa[:, None] * 8 + np.arange(8)[None, :]),
                      yy[:, None], xx[:, None]]
            d = pv[:, :4] - tt
            adx = np.abs(d)
            sl1 = np.where(adx < 1, np.float32(0.5) * d * d,
                           adx - np.float32(0.5))
            total += np.float64(sl1.sum(dtype=np.float64))
            xv = pv[:, 4]
            total += np.float64(
                (np.logaddexp(np.float32(0.0), xv) - xv).sum(dtype=np.float64))
            logits = pv[:, 5:]
            mx = logits.max(1)
            lse = mx + np.log(np.exp(logits - mx[:, None]).sum(1))
            lab = np.maximum(gtl[bb, bidx], 0)
            ce = lse - logits[np.arange(len(bb)), lab]
            total += np.float64(ce.sum(dtype=np.float64))
    return np.float32(total / B)


# ----------------------------------------------------------------------
# exact numpy fallback (no device)
# ----------------------------------------------------------------------

def _numpy_kernel(preds, anchors, gtb, gtl):
    total = np.float64(0.0)
    for si in range(3):
        anc = anchors[si]
        N = anc.shape[0]
        p_all = preds[si].transpose(0, 2, 3, 1).reshape(B, N, 8)
        for b in range(B):
            p = p_all[b]
            a = anc
            gb = gtb[b]
            lt = np.maximum(a[:, None, :2], gb[None, :, :2])
            rb = np.minimum(a[:, None, 2:], gb[None, :, 2:])
            wh = np.clip(rb - lt, 0, None)
            inter = wh[..., 0] * wh[..., 1]
            area_a = (a[:, 2] - a[:, 0]) * (a[:, 3] - a[:, 1])
            area_b = (gb[:, 2] - gb[:, 0]) * (gb[:, 3] - gb[:, 1])
            iou = inter / (area_a[:, None] + area_b[None, :] - inter
                           + np.float32(1e-9))
            best = iou.max(1)
            bidx = iou.argmax(1)
            pos = best >= POS_IOU
            neg = best < NEG_IOU
            posf = pos.astype(np.float32)
            mb = gb[bidx]
            axc = (a[:, 0] + a[:, 2]) * 0.5
            ayc = (a[:, 1] + a[:, 3]) * 0.5
            aw = np.maximum(a[:, 2] - a[:, 0], np.float32(EPS))
            ah = np.maximum(a[:, 3] - a[:, 1], np.float32(EPS))
            gx = (mb[:, 0] + mb[:, 2]) * 0.5
            gy = (mb[:, 1] + mb[:, 3]) * 0.5
            gw = np.maximum(mb[:, 2] - mb[:, 0], np.float32(EPS))
            gh = np.maximum(mb[:, 3] - mb[:, 1], np.float32(EPS))
            t = [(gx - axc) / aw, (gy - ayc) / ah,
                 np.log(gw / aw), np.log(gh / ah)]
            def sl1(x):
                ax_ = np.abs(x)
                return np.where(ax_ < 1, np.float32(0.5) * x * x,
                                ax_ - np.float32(0.5))
            total += np.float64((posf * (sl1(p[:, 0] - t[0]) + sl1(p[:, 1] - t[1])
                                 + sl1(p[:, 2] - t[2]) + sl1(p[:, 3] - t[3])
                                 )).sum(dtype=np.float64))
            x = p[:, 4]
            obj_all = (np.maximum(x, 0) - x * posf
                       + np.log1p(np.exp(-np.abs(x))))
            num_keep = NEG_RATIO * max(1, int(pos.sum()))
            neg_loss = np.where(neg, obj_all, np.float32(-1e9))
            order = np.argsort(-neg_loss, kind='stable')
            ranks = np.empty(N, np.int64)
            ranks[order] = np.arange(N)
            selected = neg & (ranks < num_keep)
            total += np.float64(
                (obj_all * (posf + selected)).sum(dtype=np.float64))
            mx = p[:, 5:].max(1)
            lse = mx + np.log(np.exp(p[:, 5:] - mx[:, None]).sum(1))
            ce = lse - p[np.arange(N), 5 + np.maximum(gtl[b][bidx], 0)]
            total += np.float64((posf * ce).sum(dtype=np.float64))
    return np.float32(total / B)


# ----------------------------------------------------------------------

def kernel(pred0, pred1, pred2, anchors0, anchors1, anchors2,
           gt_boxes, gt_labels):
    preds = [np.asarray(p, dtype=np.float32) for p in (pred0, pred1, pred2)]
    anchors = [np.asarray(a, dtype=np.float32)
               for a in (anchors0, anchors1, anchors2)]
    gtb = np.asarray(gt_boxes, dtype=np.float32)
    gtl = np.asarray(gt_labels)
    import time as _time
    tmr = {}
    try:
        t0 = _time.time()
        blobs, ok = _prep_tables(gtb)
        tmr['prep'] = _time.time() - t0
        if not ok:
            raise RuntimeError("coloring overflow; fallback")
        t0 = _time.time()
        _ensure_device()
        tmr['ensure'] = _time.time() - t0
        box = {}

        def dev():
            try:
                t = _time.time()
                box['masks'] = _run_device(blobs)
                tmr['device'] = _time.time() - t
            except Exception as e:   # noqa: BLE001
                box['err'] = e
        th = threading.Thread(target=dev, daemon=True)
        t0 = _time.time()
        th.start()
        # overlap: softplus objectness per scale (device-independent)
        sps = [np.logaddexp(np.float32(0.0), preds[si][:, 4::8])
               for si in range(3)]
        tmr['sp_overlap'] = _time.time() - t0
        th.join(timeout=90.0)
        tmr['dev_wall'] = _time.time() - t0
        if th.is_alive():
            raise RuntimeError("device run timed out")
        if 'err' in box:
            raise box['err']
        t0 = _time.time()
        r = _finish(preds, anchors, gtb, gtl, box['masks'], sps)
        tmr['finish'] = _time.time() - t0
        if os.environ.get("DETLOSS_TIMERS"):
            print("timers:", {k: round(v, 3) for k, v in tmr.items()},
                  file=sys.stderr)
        return r
    except Exception:
        traceback.print_exc()
        return _numpy_kernel(preds, anchors, gtb, gtl)


# import-time warmup (compile-cache hit + jit executable load) unless told no
if not os.environ.get("DETLOSS_NO_WARMUP"):
    try:
        _ensure_device()
    except Exception:   # noqa: BLE001
        traceback.print_exc()
